# revision 8
# baseline (speedup 1.0000x reference)
"""Causal self-attention kernel for 8 Trainium2 NeuronCores.

Problem: B=4, T=2048, C=1024, NH=16, HD=64 (fp32).

Sharding: core c = (batch b = c//2, head-group g = c%2 of 8 heads).
Per core, everything is computed in transposed layout so no on-device
transposes are needed:
  - host supplies xT = x[b].T [C, T], plus head-group-sliced/permuted
    weights (column-parallel W_attn, row-parallel W_proj)
  - qT/kT [feat, tok] via W stationary / xT moving; v [tok, feat] via
    xT stationary / W_v moving, with a fused ones-column per head so the
    attention row-sum (softmax denominator) falls out of the same matmul
  - scores are computed transposed [keys, queries] per 128-key block;
    exp on ACT; causal masking only on diagonal blocks; blocks entirely
    above the diagonal are skipped
  - y^T accumulates in PSUM over key blocks; normalized by 1/Z
  - output projection is row-parallel -> partial out^T [C, T]; pairwise
    ReduceScatter (+bias, added only on g=0 cores via host-zeroed bias)
    yields each core's final out^T rows; host concatenates + transposes.

All matmuls run as float32r (fp22, full PE rate). q/k are stored bf16 in
SBUF (the score matmul runs bf16) to fit the working set in SBUF.
"""

import numpy as np
from contextlib import ExitStack

import concourse.bass as bass
import concourse.tile as tile
import concourse.mybir as mybir
from concourse import bacc
from concourse.bass_utils import run_bass_kernel_spmd

B, C, NH, HD = 4, 1024, 16, 64
NCORES = 8
NP = 4              # head pairs per core (8 heads)
QC = 512            # query-chunk (free dim of most matmuls)
KB = 128            # key block (partition dim of score blocks)
CCH = C // 128      # 8 contraction chunks
FP32 = mybir.dt.float32
FP32R = mybir.dt.float32r
BF16 = mybir.dt.bfloat16
EXP = mybir.ActivationFunctionType.Exp
GROUPS = [[0, 1], [2, 3], [4, 5], [6, 7]]


def build_program(T=2048, mode="full"):
    nqc = T // QC
    nc = bacc.Bacc("TRN2", target_bir_lowering=False, debug=False,
                   num_devices=NCORES)

    xt_d = nc.dram_tensor("xt", [C, T], FP32, kind="ExternalInput").ap()
    wqk_d = nc.dram_tensor("wqk", [C, C], FP32, kind="ExternalInput").ap()
    bqk_d = nc.dram_tensor("bqk", [C], FP32, kind="ExternalInput").ap()
    wv_d = nc.dram_tensor("wv", [C, 512], FP32, kind="ExternalInput").ap()
    bv_d = nc.dram_tensor("bv", [512], FP32, kind="ExternalInput").ap()
    wp_d = nc.dram_tensor("wp", [512, C], FP32, kind="ExternalInput").ap()
    bp_d = nc.dram_tensor("bp", [C], FP32, kind="ExternalInput").ap()
    mask_d = nc.dram_tensor("mask", [128, QC + 384], FP32, kind="ExternalInput").ap()
    out_d = nc.dram_tensor("out_t", [512, T], FP32, kind="ExternalOutput").ap()

    with tile.TileContext(nc) as tc, ExitStack() as ctx:
        resid = ctx.enter_context(tc.tile_pool(name="resid", bufs=1))
        xtp = ctx.enter_context(tc.tile_pool(name="xtp", bufs=2))
        qp = ctx.enter_context(tc.tile_pool(name="qp", bufs=2))
        yp = ctx.enter_context(tc.tile_pool(name="yp", bufs=1))
        ep = ctx.enter_context(tc.tile_pool(name="ep", bufs=3))
        sm = ctx.enter_context(tc.tile_pool(name="sm", bufs=2))
        op = ctx.enter_context(tc.tile_pool(name="op", bufs=2))
        dram = ctx.enter_context(tc.tile_pool(name="dram", bufs=1, space="DRAM"))
        ps_acc = ctx.enter_context(tc.tile_pool(name="ps_acc", bufs=2, space="PSUM"))
        ps_s = ctx.enter_context(tc.tile_pool(name="ps_s", bufs=2, space="PSUM"))
        ps_y = ctx.enter_context(tc.tile_pool(name="ps_y", bufs=2, space="PSUM"))

        # ---- residents: weights, biases, masks, k/v/y accumulator tiles
        wqk_sb = resid.tile([128, CCH, 8, 128], FP32R, name="wqk_sb")
        for cc in range(CCH):
            nc.sync.dma_start(out=wqk_sb[:, cc], in_=wqk_d[cc * 128:(cc + 1) * 128, :].rearrange("p (f n) -> p f n", f=8).bitcast(FP32R))
        wv_sb = resid.tile([128, CCH, 512], FP32R, name="wv_sb")
        for cc in range(CCH):
            nc.sync.dma_start(out=wv_sb[:, cc], in_=wv_d[cc * 128:(cc + 1) * 128, :].bitcast(FP32R))
        wp_sb = resid.tile([128, NP, 8, 128], FP32R, name="wp_sb")
        for p in range(NP):
            nc.sync.dma_start(out=wp_sb[:, p], in_=wp_d[p * 128:(p + 1) * 128, :].rearrange("p (f n) -> p f n", f=8).bitcast(FP32R))

        bqk_sb = resid.tile([128, 8], FP32, name="bqk_sb")
        nc.sync.dma_start(out=bqk_sb, in_=bqk_d.rearrange("(f p) -> p f", p=128))
        bp_sb = resid.tile([128, 8], FP32, name="bp_sb")
        nc.sync.dma_start(out=bp_sb, in_=bp_d.rearrange("(f p) -> p f", p=128))
        # bv_bc[:, l, 0:64] = b_v (broadcast over partitions); [:, l, 64] = 1.0
        # (the ones column rides along into v so the attention row-sum Z comes
        #  out of the same av matmul)
        bv_bc = resid.tile([128, 8, HD + 1], FP32, name="bv_bc")
        nc.sync.dma_start(
            out=bv_bc[:, :, 0:HD],
            in_=bv_d.rearrange("(l d) -> l d", l=8).partition_broadcast(128))
        nc.vector.memset(bv_bc[:, :, HD:HD + 1], 1.0)

        # causal mask M[p, u] = 1.0 iff p <= u - 384 (host-generated; slice per
        # diag offset r: mask[:, 384-128r : 896-128r] = (key_p <= query_n-128r))
        mask = resid.tile([128, QC + 384], FP32R, name="mask")
        nc.sync.dma_start(out=mask, in_=mask_d.bitcast(FP32R))

        ksb = [resid.tile([128, T], BF16, name=f"ksb{p}") for p in range(NP)]
        vsb = [resid.tile([128, 8, HD + 1], FP32R, name=f"vsb{tb}")
               for tb in range(T // 128)]

        for qc in range(nqc):
            # ---- load this query-chunk's slice of xT: [C, QC]
            xt_sb = xtp.tile([128, CCH, QC], FP32R, name="xt_sb")
            for cc in range(CCH):
                nc.sync.dma_start(
                    out=xt_sb[:, cc],
                    in_=xt_d[cc * 128:(cc + 1) * 128, qc * QC:(qc + 1) * QC].bitcast(FP32R))

            # ---- v for the 4 new t-blocks (t in this chunk)
            for j in range(QC // 128):
                tb = qc * (QC // 128) + j
                pv = ps_acc.tile([128, 512], FP32, name="pv")
                for cc in range(CCH):
                    nc.tensor.matmul(
                        out=pv,
                        lhsT=xt_sb[:, cc, j * 128:(j + 1) * 128],
                        rhs=wv_sb[:, cc],
                        start=(cc == 0), stop=(cc == CCH - 1))
                nc.vector.tensor_copy(vsb[tb][:, :, HD:HD + 1],
                                      bv_bc[:, :, HD:HD + 1])
                nc.vector.tensor_add(
                    vsb[tb][:, :, 0:HD],
                    pv.rearrange("p (l d) -> p l d", l=8),
                    bv_bc[:, :, 0:HD])

            # ---- q,k for this query-chunk: feat chunk f = 2p+isK
            q_sb = [None] * NP
            for p in range(NP):
                q_sb[p] = qp.tile([128, QC], BF16, name=f"qsb{p}", tag=f"qsb{p}")
            for f in range(8):
                pqk = ps_acc.tile([128, QC], FP32, name="pqk", tag="pv")
                for cc in range(CCH):
                    nc.tensor.matmul(
                        out=pqk,
                        lhsT=wqk_sb[:, cc, f],
                        rhs=xt_sb[:, cc],
                        start=(cc == 0), stop=(cc == CCH - 1))
                p, isk = f // 2, f % 2
                dst = (ksb[p][:, qc * QC:(qc + 1) * QC] if isk else q_sb[p])
                nc.vector.tensor_scalar_add(dst, pqk, bqk_sb[:, f:f + 1])

            # ---- attention for this query-chunk
            y_qc = [None] * NP
            for p in range(NP):
                y_qc[p] = yp.tile([128, QC], FP32R, name=f"y{p}", tag=f"y{p}")
                njb = 4 * (qc + 1)
                yps = [ps_y.tile([HD + 1, QC], FP32, name=f"yps{e}", tag="yps")
                       for e in (0, 1)]
                for jb in range(njb):
                    sps = ps_s.tile([128, 2 * QC], FP32, name="sps")
                    for e in (0, 1):
                        nc.tensor.matmul(
                            out=sps[:, e * QC:(e + 1) * QC],
                            lhsT=ksb[p][e * HD:(e + 1) * HD, jb * KB:(jb + 1) * KB],
                            rhs=q_sb[p][e * HD:(e + 1) * HD, :],
                            start=True, stop=True)
                    esb = ep.tile([128, 2 * QC], FP32R, name="esb")
                    nc.scalar.activation(out=esb, in_=sps, func=EXP, scale=0.125)
                    r = jb - 4 * qc
                    if r >= 0:
                        msl = mask[:, 384 - 128 * r: 384 - 128 * r + QC]
                        for e in (0, 1):
                            nc.vector.tensor_mul(
                                esb[:, e * QC:(e + 1) * QC],
                                esb[:, e * QC:(e + 1) * QC], msl)
                    esrc = esb
                    for e in (0, 1):
                        nc.tensor.matmul(
                            out=yps[e],
                            lhsT=vsb[jb][:, 2 * p + e, :],
                            rhs=esrc[:, e * QC:(e + 1) * QC],
                            start=(jb == 0), stop=(jb == njb - 1))
                for e in (0, 1):
                    rz = sm.tile([1, QC], FP32, name="rz")
                    nc.vector.reciprocal(rz, yps[e][HD:HD + 1, :])
                    rz_dr = dram.tile([QC], FP32, name="rzd", tag="rzd", bufs=4)
                    nc.sync.dma_start(out=rz_dr, in_=rz)
                    rzb = sm.tile([HD, QC], FP32, name="rzb")
                    nc.sync.dma_start(out=rzb, in_=rz_dr.partition_broadcast(HD))
                    nc.vector.tensor_mul(
                        y_qc[p][e * HD:(e + 1) * HD, :], yps[e][0:HD, :], rzb)

            # ---- output projection (partial over this core's 512 channels)
            cc_in = nc.dram_tensor(f"ccin{qc}", [C, QC], FP32).ap()
            cc_out = nc.dram_tensor(f"ccout{qc}", [512, QC], FP32).ap()
            for oc in range(8):
                pp = ps_acc.tile([128, QC], FP32, name="pp", tag="pv")
                for p in range(NP):
                    nc.tensor.matmul(
                        out=pp,
                        lhsT=wp_sb[:, p, oc],
                        rhs=y_qc[p],
                        start=(p == 0), stop=(p == NP - 1))
                po = op.tile([128, QC], FP32, name="po")
                nc.vector.tensor_scalar_add(po, pp, bp_sb[:, oc:oc + 1])
                nc.sync.dma_start(out=cc_in[oc * 128:(oc + 1) * 128, :], in_=po)
            if mode == "nors":
                nc.sync.dma_start(out=out_d[:, qc * QC:(qc + 1) * QC],
                                  in_=cc_in[0:512, :])
            else:
                nc.gpsimd.collective_compute(
                    "ReduceScatter", mybir.AluOpType.add, replica_groups=GROUPS,
                    ins=[cc_in[:]], outs=[cc_out[:]])
                nc.sync.dma_start(out=out_d[:, qc * QC:(qc + 1) * QC], in_=cc_out[:])

    nc.compile()
    return nc


def shard_inputs(x, W_attn, b_attn, W_proj, b_proj):
    T = x.shape[1]
    in_maps = []
    for c in range(NCORES):
        b, g = c // 2, c % 2
        xt = np.ascontiguousarray(x[b].T.astype(np.float32))
        # w_qk columns: feat chunk f = 2p+isK holds q (isK=0) or k (isK=1)
        # features of heads (8g+2p, 8g+2p+1)
        qk_idx = []
        for f in range(8):
            p, isk = f // 2, f % 2
            for e in (0, 1):
                h = 8 * g + 2 * p + e
                base = isk * C + h * HD
                qk_idx.append(np.arange(base, base + HD))
        qk_idx = np.concatenate(qk_idx)
        v_idx = np.concatenate(
            [np.arange(2 * C + (8 * g + l) * HD, 2 * C + (8 * g + l) * HD + HD)
             for l in range(8)])
        p_idx = np.concatenate(
            [np.arange((8 * g + l) * HD, (8 * g + l) * HD + HD)
             for l in range(8)])
        u = np.arange(QC + 384)[None, :]
        p_ = np.arange(128)[:, None]
        mask_np = (p_ <= u - 384).astype(np.float32)
        in_maps.append({
            "mask": mask_np,
            "xt": xt,
            "wqk": np.ascontiguousarray(W_attn[:, qk_idx].astype(np.float32)),
            "bqk": np.ascontiguousarray(b_attn[qk_idx].astype(np.float32)),
            "wv": np.ascontiguousarray(W_attn[:, v_idx].astype(np.float32)),
            "bv": np.ascontiguousarray(b_attn[v_idx].astype(np.float32)),
            "wp": np.ascontiguousarray(W_proj[p_idx, :].astype(np.float32)),
            "bp": (b_proj.astype(np.float32) if g == 0
                   else np.zeros(C, np.float32)),
        })
    return in_maps


def assemble_output(results, T):
    out = np.empty((B, T, C), np.float32)
    for b in range(B):
        top = results[2 * b]["out_t"]         # out[b].T rows 0:512
        bot = results[2 * b + 1]["out_t"]     # rows 512:1024
        out[b] = np.concatenate([top, bot], axis=0).T
    return out


_PROG = {}


def _get_program(T, mode="full"):
    key = (T, mode)
    if key not in _PROG:
        _PROG[key] = build_program(T, mode)
    return _PROG[key]


def run_sharded(inputs, trace=False, mode="full"):
    """Returns (output [B,T,C], BassKernelResults)."""
    x = np.asarray(inputs["x"])
    T = x.shape[1]
    nc = _get_program(T, mode)
    in_maps = shard_inputs(x, np.asarray(inputs["W_attn"]),
                           np.asarray(inputs["b_attn"]),
                           np.asarray(inputs["W_proj"]),
                           np.asarray(inputs["b_proj"]))
    res = run_bass_kernel_spmd(nc, in_maps, list(range(NCORES)), trace=trace)
    return assemble_output(res.results, T), res


def kernel(**inputs):
    out, _ = run_sharded(inputs)
    return out
